# revision 1
# baseline (speedup 1.0000x reference)
"""Causal self-attention (B=4, S=2048, H=2048, 16 heads) on 8 Trainium2 NeuronCores.

Sharding: DP4 over batch x TP2 over heads. Core c handles batch c//2 and head
half c%2 (8 heads of 128 dims). fp16 matmul operands throughout (PSUM always
accumulates fp32); fp16 halves SBUF so the projection and attention phases
coexist and interleave, and halves DMA traffic. Per core:
  phase 1: V ([s,d] layout) and Q^T,K^T ([d,s] layout) projections, bounced to
           DRAM scratch (fp16). x^T loaded in two column-halves so the first
           projections start after 8MB, not 16MB.
  phase 2: per head, causal flash-style attention in the transposed layout
           (scores^T [k,q]): softmax denominators via ones-matmul, PV without
           transposes, exp batched 2 score tiles per ACTIVATE. Unnormalized
           ctx^T parked in SBUF; denominator strips collected in DRAM, one
           batched reciprocal per 2-head group (DRAM roundtrip to repack rows
           to partition 0), PE-broadcast, normalize into fp16. Emission
           interleaves attention(h) between later projections so PE gaps fill.
  phase 3: four pairwise fp16 AllGathers (one per 2-head group), launched as
           each group finishes so they overlap the remaining attention.
  phase 4: fp16 output projection from 16 ctx row-strips; the o-range is split
           across the pair via the per-core Wo slice (no program divergence),
           bias folded on host (bo_eff = bo + Wo @ bv; bv dropped from the V
           projection since softmax rows sum to 1). PSUM accumulation ordered
           chunk 0..3 so early chunks start before the last AllGather lands.
Host assembles out[b, :, o_half] = per-core out [s, o_half].
"""

import math
import sys

if "/opt/trn_rl_repo" not in sys.path:
    sys.path.insert(0, "/opt/trn_rl_repo")

import numpy as np

B, S, HID = 4, 2048, 2048
HEADS, D = 16, 128
HH = HEADS // 2          # heads per core
HHID = HH * D            # 1024, per-core head-span of hidden
KT = HID // 128          # 16 contraction tiles of 128
NB = S // 512            # 4 free-dim blocks of 512
N_CORES = 8
NCHUNK = 4               # ctx-exchange chunks (2 heads each)

_CACHED = {}


def _build_program():
    import concourse.tile as tile
    import concourse.mybir as mybir
    from concourse import bacc
    from concourse._compat import get_trn_type

    F32 = mybir.dt.float32
    F16 = mybir.dt.float16
    Exp = mybir.ActivationFunctionType.Exp
    Copy = mybir.ActivationFunctionType.Copy

    nc = bacc.Bacc(
        get_trn_type() or "TRN2",
        target_bir_lowering=False,
        debug=False,
        enable_asserts=False,
        num_devices=N_CORES,
    )

    def din(name, shape, dt=F16):
        return nc.dram_tensor(name, shape, dt, kind="ExternalInput").ap()

    xT = din("xT", [HID, S])          # x[b].T, fp16
    wqT = din("wqT", [HID, HHID])     # Wq.T columns for this core's heads
    wkT = din("wkT", [HID, HHID])
    wvT = din("wvT", [HID, HHID])
    woT = din("woT", [HID, HHID])     # Wo.T columns for this core's o-half
    bq = din("bq", [128, HH], F32)    # bq[h*128+p] at [p, h]
    bk = din("bk", [128, HH], F32)
    bo = din("bo", [1, HHID], F16)    # bo_eff slice for this core's o-half
    masks = din("masks", [4, 128, 512])
    out = nc.dram_tensor("out", [S, HHID], F32, kind="ExternalOutput").ap()

    inv_sqrt_d = float(1.0 / math.sqrt(D))

    with tile.TileContext(nc) as tc, \
         nc.allow_low_precision(reason="fp16 operand pipeline"):
        with tc.tile_pool(name="const", bufs=1) as constp, \
             tc.tile_pool(name="dram", bufs=1, space="DRAM") as dramp:
            # DRAM scratch (fp16 except denominator strips)
            qTd = dramp.tile([HHID, S], F16, tag="qTd")
            kTd = dramp.tile([HHID, S], F16, tag="kTd")
            vd = dramp.tile([S, HHID], F16, tag="vd")
            dden_d = [dramp.tile([8, 512], F32, tag=f"dden{c}",
                                 name=f"dden{c}") for c in range(NCHUNK)]
            rden_d = [dramp.tile([8, 512], F16, tag=f"rden{c}",
                                 name=f"rden{c}") for c in range(NCHUNK)]
            ctx_send = [dramp.tile([256, S], F16, tag=f"ctxs{c}",
                                   name=f"ctxs{c}") for c in range(NCHUNK)]
            ctx_recv = [dramp.tile([512, S], F16, tag=f"ctxr{c}",
                                   name=f"ctxr{c}") for c in range(NCHUNK)]

            # constants
            ones_col = constp.tile([128, 1], F16, tag="ones_col")
            nc.vector.memset(ones_col, 1.0)
            ones_row = constp.tile([1, 128], F16, tag="ones_row")
            nc.vector.memset(ones_row, 1.0)
            mask_t = []
            for r in range(4):
                mt = constp.tile([128, 512], F16, tag=f"mask{r}",
                                 name=f"mask{r}")
                nc.sync.dma_start(out=mt, in_=masks[r])
                mask_t.append(mt)
            bq_sb = constp.tile([128, HH], F32, tag="bq_sb")
            nc.sync.dma_start(out=bq_sb, in_=bq)
            bk_sb = constp.tile([128, HH], F32, tag="bk_sb")
            nc.sync.dma_start(out=bk_sb, in_=bk)
            bo_sb = constp.tile([1, HHID], F16, tag="bo_sb")
            nc.sync.dma_start(out=bo_sb, in_=bo)

            with tc.tile_pool(name="xk", bufs=2 * KT) as xp, \
                 tc.tile_pool(name="p1s", bufs=4) as sp, \
                 tc.tile_pool(name="p1w", bufs=16) as wp, \
                 tc.tile_pool(name="p1v", bufs=4) as vp, \
                 tc.tile_pool(name="p1wv", bufs=16) as wvp, \
                 tc.tile_pool(name="p2qk", bufs=2) as qkp, \
                 tc.tile_pool(name="p2v", bufs=18) as v4p, \
                 tc.tile_pool(name="p2et", bufs=4) as etp, \
                 tc.tile_pool(name="p2cu", bufs=10) as cup, \
                 tc.tile_pool(name="p2c", bufs=2) as cp, \
                 tc.tile_pool(name="p2d", bufs=2) as dp, \
                 tc.tile_pool(name="ps1", bufs=2, space="PSUM") as pp, \
                 tc.tile_pool(name="ps2s", bufs=2, space="PSUM") as pps, \
                 tc.tile_pool(name="ps2c", bufs=1, space="PSUM") as ppc, \
                 tc.tile_pool(name="ps2d", bufs=1, space="PSUM") as ppd:
                # x^T in two column-halves: [k][half] tiles of [128, 1024]
                xk = [[None, None] for _ in range(KT)]
                for half in range(2):
                    for k in range(KT):
                        t = xp.tile([128, 1024], F16, tag="xk",
                                    name=f"xk{k}_{half}")
                        nc.sync.dma_start(
                            out=t,
                            in_=xT[k * 128:(k + 1) * 128,
                                   half * 1024:(half + 1) * 1024])
                        xk[k][half] = t

                def xslice(k, lo, size):
                    half, off = lo // 1024, lo % 1024
                    return xk[k][half][:, off:off + size]

                def v_proj(g):
                    wvt = []
                    for k in range(KT):
                        w = wvp.tile([128, 512], F16, tag="wv",
                                     name=f"wv{g}_{k}")
                        nc.sync.dma_start(
                            out=w,
                            in_=wvT[k * 128:(k + 1) * 128,
                                    g * 512:(g + 1) * 512])
                        wvt.append(w)
                    for m in range(KT):
                        ps = pp.tile([128, 512], F32, tag="ps1",
                                     name=f"psv{g}_{m}")
                        for k in range(KT):
                            nc.tensor.matmul(
                                ps, xslice(k, m * 128, 128), wvt[k],
                                start=(k == 0), stop=(k == KT - 1))
                        vsb = vp.tile([128, 512], F16, tag="vout",
                                      name=f"v{g}_{m}")
                        nc.vector.tensor_copy(out=vsb, in_=ps)
                        nc.sync.dma_start(
                            out=vd[m * 128:(m + 1) * 128,
                                   g * 512:(g + 1) * 512], in_=vsb)

                def qk_proj(h):
                    for wT, dst, bias_sb, pname in (
                        (wqT, qTd, bq_sb, "q"),
                        (wkT, kTd, bk_sb, "k"),
                    ):
                        wt = []
                        for k in range(KT):
                            w = wp.tile([128, 128], F16, tag="w",
                                        name=f"w{pname}{h}_{k}")
                            nc.sync.dma_start(
                                out=w,
                                in_=wT[k * 128:(k + 1) * 128,
                                       h * 128:(h + 1) * 128])
                            wt.append(w)
                        for n in range(NB):
                            ps = pp.tile([128, 512], F32, tag="ps1",
                                         name=f"ps{pname}{h}_{n}")
                            for k in range(KT):
                                nc.tensor.matmul(
                                    ps, wt[k], xslice(k, n * 512, 512),
                                    start=(k == 0), stop=(k == KT - 1))
                            osb = sp.tile([128, 512], F16, tag="projout",
                                          name=f"o{pname}{h}_{n}")
                            nc.vector.tensor_scalar_add(
                                osb, ps, bias_sb[:, h:h + 1])
                            nc.sync.dma_start(
                                out=dst[h * 128:(h + 1) * 128,
                                        n * 512:(n + 1) * 512], in_=osb)

                v4 = [None, None]
                ctx_u = {}

                def load_v4(g):
                    tiles = []
                    for k in range(KT):
                        t = v4p.tile([128, 512], F16, tag="v4",
                                     name=f"v4_{g}_{k}")
                        nc.sync.dma_start(
                            out=t,
                            in_=vd[k * 128:(k + 1) * 128,
                                   g * 512:(g + 1) * 512])
                        tiles.append(t)
                    v4[g] = tiles

                def attention(h):
                    g, sub = h // 4, h % 4
                    qh = qkp.tile([128, S], F16, tag="qh", name=f"qh{h}")
                    nc.sync.dma_start(out=qh,
                                      in_=qTd[h * 128:(h + 1) * 128, :])
                    kh = qkp.tile([128, S], F16, tag="kh", name=f"kh{h}")
                    nc.sync.dma_start(out=kh,
                                      in_=kTd[h * 128:(h + 1) * 128, :])
                    for qb in range(NB):
                        kept = min(4 * qb + 4, KT)
                        ctx_ps = ppc.tile([128, 512], F32, tag="ctxps",
                                          name=f"cps{h}_{qb}")
                        den_ps = ppd.tile([1, 512], F32, tag="denps",
                                          name=f"dps{h}_{qb}")
                        for kt0 in range(0, kept, 2):
                            sps = pps.tile([128, 1024], F32, tag="sps",
                                           name=f"sps{h}_{qb}_{kt0}")
                            for i in range(2):
                                nc.tensor.matmul(
                                    sps[:, i * 512:(i + 1) * 512],
                                    kh[:, (kt0 + i) * 128:(kt0 + i + 1) * 128],
                                    qh[:, qb * 512:(qb + 1) * 512],
                                    start=True, stop=True)
                            et = etp.tile([128, 1024], F16, tag="et",
                                          name=f"et{h}_{qb}_{kt0}")
                            nc.scalar.activation(out=et, in_=sps, func=Exp,
                                                 scale=inv_sqrt_d)
                            for i in range(2):
                                kt = kt0 + i
                                ets = et[:, i * 512:(i + 1) * 512]
                                r = kt - 4 * qb
                                if r >= 0:
                                    nc.vector.tensor_mul(ets, ets, mask_t[r])
                                nc.tensor.matmul(
                                    den_ps, ones_col, ets,
                                    start=(kt == 0), stop=(kt == kept - 1))
                                nc.tensor.matmul(
                                    ctx_ps,
                                    v4[g][kt][:, sub * 128:(sub + 1) * 128],
                                    ets,
                                    start=(kt == 0), stop=(kt == kept - 1))
                        row = (h % 2) * 4 + qb
                        den_sb = dp.tile([1, 512], F32, tag="densb",
                                         name=f"den{h}_{qb}", bufs=3)
                        nc.vector.tensor_copy(out=den_sb, in_=den_ps)
                        nc.sync.dma_start(
                            out=dden_d[h // 2][row:row + 1, :], in_=den_sb)
                        cu = cup.tile([128, 512], F32, tag="cu",
                                      name=f"cu{h}_{qb}")
                        nc.scalar.activation(out=cu, in_=ctx_ps, func=Copy)
                        ctx_u[(h, qb)] = cu
                    if h % 2 == 1:
                        c = h // 2
                        dpack = dp.tile([8, 512], F32, tag="dpack",
                                        name=f"dpack{c}", bufs=2)
                        nc.sync.dma_start(out=dpack, in_=dden_d[c])
                        rpack = dp.tile([8, 512], F16, tag="rpack",
                                        name=f"rpack{c}", bufs=2)
                        nc.vector.reciprocal(out=rpack, in_=dpack)
                        nc.sync.dma_start(out=rden_d[c], in_=rpack)
                        rstrip = dp.tile([1, 8, 512], F16, tag="rstrip",
                                         name=f"rstrip{c}", bufs=2)
                        nc.sync.dma_start(
                            out=rstrip,
                            in_=rden_d[c].rearrange("(o r) c -> o r c", o=1))
                        for hh in (h - 1, h):
                            ct16 = cp.tile([128, S], F16, tag="ctxh",
                                           name=f"ctxh{hh}")
                            for qb in range(NB):
                                row = (hh % 2) * 4 + qb
                                dbc_ps = pp.tile([128, 512], F32, tag="ps1",
                                                 name=f"dbc{hh}_{qb}")
                                nc.tensor.matmul(
                                    dbc_ps, ones_row, rstrip[:, row, :],
                                    start=True, stop=True)
                                nc.vector.tensor_mul(
                                    ct16[:, qb * 512:(qb + 1) * 512],
                                    ctx_u[(hh, qb)], dbc_ps)
                            nc.sync.dma_start(
                                out=ctx_send[c][(hh % 2) * 128:
                                                (hh % 2) * 128 + 128, :],
                                in_=ct16)
                        nc.gpsimd.collective_compute(
                            "AllGather",
                            mybir.AluOpType.bypass,
                            replica_groups=[[0, 1], [2, 3], [4, 5], [6, 7]],
                            ins=[ctx_send[c].opt()],
                            outs=[ctx_recv[c].opt()],
                        )

                # interleaved emission: projections feed attention per head
                qk_proj(0)
                qk_proj(1)
                v_proj(0)
                load_v4(0)
                qk_proj(2)
                attention(0)
                qk_proj(3)
                attention(1)
                v_proj(1)
                qk_proj(4)
                attention(2)
                qk_proj(5)
                attention(3)
                load_v4(1)
                qk_proj(6)
                attention(4)
                qk_proj(7)
                attention(5)
                attention(6)
                attention(7)

            # ---------------- phase 4: output projection ----------------
            korder = []
            for c in range(NCHUNK):
                korder += [(c, 0, 2 * c), (c, 128, 2 * c + 1),
                           (c, 256, 8 + 2 * c), (c, 384, 8 + 2 * c + 1)]
            with tc.tile_pool(name="p4wo", bufs=32) as wop, \
                 tc.tile_pool(name="p4ct", bufs=16) as ctp, \
                 tc.tile_pool(name="p4o", bufs=3) as op_, \
                 tc.tile_pool(name="p4b", bufs=1) as bp4, \
                 tc.tile_pool(name="ps4", bufs=4, space="PSUM") as pp4:
                # broadcast bo across partitions via ones outer product
                bo_bc = bp4.tile([128, HHID], F32, tag="bo_bc")
                for n in range(HHID // 512):
                    bps = pp4.tile([128, 512], F32, tag="ps4", name=f"bps{n}")
                    nc.tensor.matmul(bps, ones_row,
                                     bo_sb[:, n * 512:(n + 1) * 512],
                                     start=True, stop=True)
                    nc.vector.tensor_copy(out=bo_bc[:, n * 512:(n + 1) * 512],
                                          in_=bps)
                wo = {}
                ct = []
                for ki, (c, off, gk) in enumerate(korder):
                    t = ctp.tile([128, S], F16, tag="ct", name=f"ct{gk}")
                    nc.sync.dma_start(out=t, in_=ctx_recv[c][off:off + 128, :])
                    ct.append(t)
                    for n in range(HHID // 512):
                        w = wop.tile([128, 512], F16, tag="wo",
                                     name=f"wo{gk}_{n}")
                        nc.sync.dma_start(
                            out=w,
                            in_=woT[gk * 128:(gk + 1) * 128,
                                    n * 512:(n + 1) * 512])
                        wo[(ki, n)] = w
                for m in range(S // 128):
                    osb = op_.tile([128, HHID], F32, tag="osb", name=f"osb{m}")
                    for n in range(HHID // 512):
                        ps = pp4.tile([128, 512], F32, tag="ps4",
                                      name=f"ps4_{m}_{n}")
                        for ki in range(KT):
                            nc.tensor.matmul(
                                ps, ct[ki][:, m * 128:(m + 1) * 128],
                                wo[(ki, n)],
                                start=(ki == 0), stop=(ki == KT - 1))
                        nc.vector.tensor_add(
                            osb[:, n * 512:(n + 1) * 512], ps,
                            bo_bc[:, n * 512:(n + 1) * 512])
                    nc.sync.dma_start(out=out[m * 128:(m + 1) * 128, :],
                                      in_=osb)

    nc.compile()
    return nc


def _get_nc():
    if "nc" not in _CACHED:
        _CACHED["nc"] = _build_program()
    return _CACHED["nc"]


def _make_masks():
    i = np.arange(128)[:, None]
    j = np.arange(512)[None, :]
    return np.stack(
        [((j - i) >= 128 * r).astype(np.float16) for r in range(4)], axis=0)


def _make_in_maps(inputs):
    x = np.ascontiguousarray(np.asarray(inputs["x"], dtype=np.float32))
    Wq = np.asarray(inputs["Wq"], dtype=np.float32)
    Wk = np.asarray(inputs["Wk"], dtype=np.float32)
    Wv = np.asarray(inputs["Wv"], dtype=np.float32)
    Wo = np.asarray(inputs["Wo"], dtype=np.float32)
    bq = np.asarray(inputs["bq"], dtype=np.float32)
    bk = np.asarray(inputs["bk"], dtype=np.float32)
    bv = np.asarray(inputs["bv"], dtype=np.float32)
    bo = np.asarray(inputs["bo"], dtype=np.float32)

    bo_eff = bo + Wo @ bv
    masks = _make_masks()
    WqT = np.ascontiguousarray(Wq.T)
    WkT = np.ascontiguousarray(Wk.T)
    WvT = np.ascontiguousarray(Wv.T)
    WoT = np.ascontiguousarray(Wo.T)

    in_maps = []
    for c in range(N_CORES):
        b, hf = c // 2, c % 2
        sl = slice(hf * HHID, (hf + 1) * HHID)
        in_maps.append({
            "xT": np.ascontiguousarray(x[b].T).astype(np.float16),
            "wqT": np.ascontiguousarray(WqT[:, sl]).astype(np.float16),
            "wkT": np.ascontiguousarray(WkT[:, sl]).astype(np.float16),
            "wvT": np.ascontiguousarray(WvT[:, sl]).astype(np.float16),
            "woT": np.ascontiguousarray(WoT[:, sl]).astype(np.float16),
            "bq": np.ascontiguousarray(bq[sl].reshape(HH, 128).T),
            "bk": np.ascontiguousarray(bk[sl].reshape(HH, 128).T),
            "bo": bo_eff[sl].reshape(1, HHID).astype(np.float16),
            "masks": masks,
        })
    return in_maps


def kernel(**inputs):
    from concourse.bass_utils import run_bass_kernel_spmd

    in_maps = _make_in_maps(inputs)
    nc = _get_nc()
    res = run_bass_kernel_spmd(nc, in_maps, list(range(N_CORES)))

    out = np.empty((B, S, HID), dtype=np.float32)
    for c in range(N_CORES):
        b, hf = c // 2, c % 2
        out[b, :, hf * HHID:(hf + 1) * HHID] = res.results[c]["out"]
    return out



# revision 7
# speedup vs baseline: 1.1734x; 1.1734x over previous
"""Causal self-attention (B=4, S=2048, H=2048, 16 heads) on 8 Trainium2 NeuronCores.

Sharding: DP4 over batch x TP2 over heads. Core c handles batch c//2 and head
half c%2 (8 heads of 128 dims). fp16 matmul operands throughout (PSUM always
accumulates fp32); fp16 halves SBUF so the projection and attention phases
coexist and interleave, and halves DMA traffic. Per core:
  phase 1: V ([s,d] layout) and Q^T,K^T ([d,s] layout) projections, bounced to
           DRAM scratch (fp16). Head-0/1 q,k weights are DMA'd BEFORE the x
           tiles so the first matmul isn't stuck behind 8MB of x traffic.
  phase 2: per head, causal flash-style attention in the transposed layout
           (scores^T [k,q]): softmax denominators via ones-matmul packed into
           PE col-groups 0 and 64 (two concurrent M=1 matmuls), PV without
           transposes. Per 2-tile pair: scores -> exp in two 512-wide
           ACTIVATEs -> mask -> [next pair's scores] -> ctx, ctx, den-pair.
           The one-pair software-pipeline lookahead keeps PE fed while
           exp/mask latency drains. Unnormalized ctx^T parked in SBUF;
           even/odd-kt denominator partials land on PSUM partitions 0/64,
           are collected in DRAM, summed + one batched reciprocal per 2-head
           group, PE-broadcast, normalize into fp16.
  phase 3: four pairwise fp16 AllGathers (one per 2-head group), launched as
           each group finishes so they overlap the remaining attention.
  phase 4: fp16 output projection from 16 ctx row-strips, two passes: pass A
           accumulates the 12 strips from AllGather chunks 0-2 (+bias) into
           SBUF while chunk 3 is still in flight; pass B adds the last 4
           strips. The o-range is split across the pair via the per-core Wo
           slice; bias folded on host (bo_eff = bo + Wo @ bv).
Host assembles out[b, :, o_half] = per-core out [s, o_half].
"""

import math
import sys

if "/opt/trn_rl_repo" not in sys.path:
    sys.path.insert(0, "/opt/trn_rl_repo")

import numpy as np

B, S, HID = 4, 2048, 2048
HEADS, D = 16, 128
HH = HEADS // 2          # heads per core
HHID = HH * D            # 1024, per-core head-span of hidden
KT = HID // 128          # 16 contraction tiles of 128
NB = S // 512            # 4 free-dim blocks of 512
N_CORES = 8
NCHUNK = 4               # ctx-exchange chunks (2 heads each)

_CACHED = {}


def _build_program():
    import concourse.tile as tile
    import concourse.mybir as mybir
    from concourse import bacc
    from concourse._compat import get_trn_type

    F32 = mybir.dt.float32
    F16 = mybir.dt.float16
    Exp = mybir.ActivationFunctionType.Exp

    nc = bacc.Bacc(
        get_trn_type() or "TRN2",
        target_bir_lowering=False,
        debug=False,
        enable_asserts=False,
        num_devices=N_CORES,
    )

    def din(name, shape, dt=F16):
        return nc.dram_tensor(name, shape, dt, kind="ExternalInput").ap()

    xT = din("xT", [HID, S])          # x[b].T, fp16
    wqT = din("wqT", [HID, HHID])     # Wq.T columns for this core's heads
    wkT = din("wkT", [HID, HHID])
    wvT = din("wvT", [HID, HHID])
    woT = din("woT", [HID, HHID])     # Wo.T columns for this core's o-half
    bq = din("bq", [128, HH], F32)    # bq[h*128+p] at [p, h]
    bk = din("bk", [128, HH], F32)
    bo = din("bo", [1, HHID], F16)    # bo_eff slice for this core's o-half
    masks = din("masks", [4, 128, 512])
    out = nc.dram_tensor("out", [S, HHID], F32, kind="ExternalOutput").ap()

    inv_sqrt_d = float(1.0 / math.sqrt(D))

    with tile.TileContext(nc) as tc, \
         nc.allow_low_precision(reason="fp16 operand pipeline"):
        with tc.tile_pool(name="const", bufs=1) as constp, \
             tc.tile_pool(name="dram", bufs=1, space="DRAM") as dramp:
            # DRAM scratch (fp16 except denominator strips)
            qTd = dramp.tile([HHID, S], F16, tag="qTd")
            kTd = dramp.tile([HHID, S], F16, tag="kTd")
            vd = dramp.tile([S, HHID], F16, tag="vd")
            # rows 0-7: even-kt partials, rows 8-15: odd-kt partials
            dden_d = [dramp.tile([16, 512], F32, tag=f"dden{c}",
                                 name=f"dden{c}") for c in range(NCHUNK)]
            rden_d = [dramp.tile([8, 512], F16, tag=f"rden{c}",
                                 name=f"rden{c}") for c in range(NCHUNK)]
            ctx_send = [dramp.tile([256, S], F16, tag=f"ctxs{c}",
                                   name=f"ctxs{c}") for c in range(NCHUNK)]
            ctx_recv = [dramp.tile([512, S], F16, tag=f"ctxr{c}",
                                   name=f"ctxr{c}") for c in range(NCHUNK)]

            # constants
            ones_col = constp.tile([128, 1], F16, tag="ones_col")
            nc.vector.memset(ones_col, 1.0)
            ones_row = constp.tile([1, 128], F16, tag="ones_row")
            nc.vector.memset(ones_row, 1.0)
            mask_t = []
            for r in range(4):
                mt = constp.tile([128, 512], F16, tag=f"mask{r}",
                                 name=f"mask{r}")
                nc.sync.dma_start(out=mt, in_=masks[r])
                mask_t.append(mt)
            bq_sb = constp.tile([128, HH], F32, tag="bq_sb")
            nc.sync.dma_start(out=bq_sb, in_=bq)
            bk_sb = constp.tile([128, HH], F32, tag="bk_sb")
            nc.sync.dma_start(out=bk_sb, in_=bk)
            bo_sb = constp.tile([1, HHID], F16, tag="bo_sb")
            nc.sync.dma_start(out=bo_sb, in_=bo)

            with tc.tile_pool(name="xk", bufs=2 * KT) as xp, \
                 tc.tile_pool(name="p1s", bufs=4) as sp, \
                 tc.tile_pool(name="p1w", bufs=32) as wp, \
                 tc.tile_pool(name="p1v", bufs=4) as vp, \
                 tc.tile_pool(name="p1wv", bufs=16) as wvp, \
                 tc.tile_pool(name="p2qk", bufs=4) as qkp, \
                 tc.tile_pool(name="p2v", bufs=18) as v4p, \
                 tc.tile_pool(name="p2et", bufs=3) as etp, \
                 tc.tile_pool(name="p2cu", bufs=10) as cup, \
                 tc.tile_pool(name="p2c", bufs=2) as cp, \
                 tc.tile_pool(name="p2d", bufs=2) as dp, \
                 tc.tile_pool(name="ps1", bufs=2, space="PSUM") as pp, \
                 tc.tile_pool(name="ps2s", bufs=2, space="PSUM") as pps, \
                 tc.tile_pool(name="ps2c", bufs=1, space="PSUM") as ppc, \
                 tc.tile_pool(name="ps2d", bufs=1, space="PSUM") as ppd:
                # head-0 q,k weights first: they gate the first matmuls
                prew = {("q", 0): [], ("k", 0): []}
                for pname, h, k in [(p, 0, k) for p in ("q", "k")
                                    for k in range(KT)]:
                    wT = wqT if pname == "q" else wkT
                    w = wp.tile([128, 128], F16, tag="w",
                                name=f"w{pname}{h}_{k}")
                    nc.sync.dma_start(
                        out=w,
                        in_=wT[k * 128:(k + 1) * 128,
                               h * 128:(h + 1) * 128])
                    prew[(pname, h)].append(w)

                # x^T in two column-halves: [k][half] tiles of [128, 1024]
                xk = [[None, None] for _ in range(KT)]
                for half in range(2):
                    for k in range(KT):
                        t = xp.tile([128, 1024], F16, tag="xk",
                                    name=f"xk{k}_{half}")
                        nc.sync.dma_start(
                            out=t,
                            in_=xT[k * 128:(k + 1) * 128,
                                   half * 1024:(half + 1) * 1024])
                        xk[k][half] = t

                def xslice(k, lo, size):
                    half, off = lo // 1024, lo % 1024
                    return xk[k][half][:, off:off + size]

                def v_proj(g):
                    wvt = []
                    for k in range(KT):
                        w = wvp.tile([128, 512], F16, tag="wv",
                                     name=f"wv{g}_{k}")
                        nc.sync.dma_start(
                            out=w,
                            in_=wvT[k * 128:(k + 1) * 128,
                                    g * 512:(g + 1) * 512])
                        wvt.append(w)
                    for m in range(KT):
                        ps = pp.tile([128, 512], F32, tag="ps1",
                                     name=f"psv{g}_{m}")
                        for k in range(KT):
                            nc.tensor.matmul(
                                ps, xslice(k, m * 128, 128), wvt[k],
                                start=(k == 0), stop=(k == KT - 1))
                        vsb = vp.tile([128, 512], F16, tag="vout",
                                      name=f"v{g}_{m}")
                        nc.vector.tensor_copy(out=vsb, in_=ps)
                        nc.sync.dma_start(
                            out=vd[m * 128:(m + 1) * 128,
                                   g * 512:(g + 1) * 512], in_=vsb)

                def qk_proj(h):
                    for wT, dst, bias_sb, pname in (
                        (wqT, qTd, bq_sb, "q"),
                        (wkT, kTd, bk_sb, "k"),
                    ):
                        wt = prew.pop((pname, h), None)
                        if wt is None:
                            wt = []
                            for k in range(KT):
                                w = wp.tile([128, 128], F16, tag="w",
                                            name=f"w{pname}{h}_{k}")
                                nc.sync.dma_start(
                                    out=w,
                                    in_=wT[k * 128:(k + 1) * 128,
                                           h * 128:(h + 1) * 128])
                                wt.append(w)
                        for n in range(NB):
                            ps = pp.tile([128, 512], F32, tag="ps1",
                                         name=f"ps{pname}{h}_{n}")
                            for k in range(KT):
                                nc.tensor.matmul(
                                    ps, wt[k], xslice(k, n * 512, 512),
                                    start=(k == 0), stop=(k == KT - 1))
                            osb = sp.tile([128, 512], F16, tag="projout",
                                          name=f"o{pname}{h}_{n}")
                            nc.vector.tensor_scalar_add(
                                osb, ps, bias_sb[:, h:h + 1])
                            nc.sync.dma_start(
                                out=dst[h * 128:(h + 1) * 128,
                                        n * 512:(n + 1) * 512], in_=osb)

                v4 = [None, None]
                ctx_u = {}

                def load_v4(g):
                    tiles = []
                    for k in range(KT):
                        t = v4p.tile([128, 512], F16, tag="v4",
                                     name=f"v4_{g}_{k}")
                        nc.sync.dma_start(
                            out=t,
                            in_=vd[k * 128:(k + 1) * 128,
                                   g * 512:(g + 1) * 512])
                        tiles.append(t)
                    v4[g] = tiles

                def attention(h):
                    g, sub = h // 4, h % 4
                    qh = qkp.tile([128, S], F16, tag="qh", name=f"qh{h}")
                    nc.sync.dma_start(out=qh,
                                      in_=qTd[h * 128:(h + 1) * 128, :])
                    kh = qkp.tile([128, S], F16, tag="kh", name=f"kh{h}")
                    nc.sync.dma_start(out=kh,
                                      in_=kTd[h * 128:(h + 1) * 128, :])

                    state = {}   # per-qb psum tiles

                    def emit_scores(qb, kt0):
                        # two score MMs + two 512-wide exps (+ masks)
                        sps = pps.tile([128, 1024], F32, tag="sps",
                                       name=f"sps{h}_{qb}_{kt0}")
                        for i in range(2):
                            nc.tensor.matmul(
                                sps[:, i * 512:(i + 1) * 512],
                                kh[:, (kt0 + i) * 128:(kt0 + i + 1) * 128],
                                qh[:, qb * 512:(qb + 1) * 512],
                                start=True, stop=True)
                        et = etp.tile([128, 1024], F16, tag="et",
                                      name=f"et{h}_{qb}_{kt0}")
                        for i in range(2):
                            ets = et[:, i * 512:(i + 1) * 512]
                            nc.scalar.activation(
                                out=ets, in_=sps[:, i * 512:(i + 1) * 512],
                                func=Exp, scale=inv_sqrt_d)
                            r = (kt0 + i) - 4 * qb
                            if r >= 0:
                                nc.vector.tensor_mul(ets, ets, mask_t[r])
                        return et

                    def emit_dc(qb, kt0, et, kept):
                        # ctx, ctx, then den pair on col-groups 0/64
                        if kt0 == 0:
                            state[qb] = (
                                ppc.tile([128, 512], F32, tag="ctxps",
                                         name=f"cps{h}_{qb}"),
                                ppd.tile([128, 512], F32, tag="denps",
                                         name=f"dps{h}_{qb}"),
                            )
                        ctx_ps, den_ps = state[qb]
                        for i in range(2):
                            kt = kt0 + i
                            nc.tensor.matmul(
                                ctx_ps,
                                v4[g][kt][:, sub * 128:(sub + 1) * 128],
                                et[:, i * 512:(i + 1) * 512],
                                start=(kt == 0), stop=(kt == kept - 1))
                        for i in range(2):
                            nc.tensor.matmul(
                                den_ps[64 * i:64 * i + 1, :],
                                ones_col, et[:, i * 512:(i + 1) * 512],
                                start=(kt0 == 0), stop=(kt0 == kept - 2),
                                tile_position=(0, 64 * i),
                                skip_group_check=True)
                        if kt0 == kept - 2:
                            # qb epilogue: evacuate den partials + ctx
                            row = (h % 2) * 4 + qb
                            den_sb = dp.tile([65, 512], F32, tag="densb",
                                             name=f"den{h}_{qb}", bufs=2)
                            nc.vector.tensor_copy(out=den_sb,
                                                  in_=den_ps[0:65, :])
                            nc.sync.dma_start(
                                out=dden_d[h // 2][row:row + 1, :],
                                in_=den_sb[0:1, :])
                            nc.sync.dma_start(
                                out=dden_d[h // 2][8 + row:9 + row, :],
                                in_=den_sb[64:65, :])
                            cu = cup.tile([128, 512], F32, tag="cu",
                                          name=f"cu{h}_{qb}")
                            nc.vector.tensor_copy(out=cu, in_=ctx_ps)
                            ctx_u[(h, qb)] = cu

                    units = []
                    for qb in range(NB):
                        kept = min(4 * qb + 4, KT)
                        for kt0 in range(0, kept, 2):
                            units.append((qb, kt0, kept))
                    prev = None
                    for qb, kt0, kept in units:
                        et = emit_scores(qb, kt0)
                        if prev is not None:
                            emit_dc(*prev)
                        prev = (qb, kt0, et, kept)
                    emit_dc(*prev)

                    if h % 2 == 1:
                        c = h // 2
                        dpk = dp.tile([8, 2, 512], F32, tag="dpack",
                                      name=f"dpack{c}", bufs=1)
                        nc.sync.dma_start(
                            out=dpk,
                            in_=dden_d[c].rearrange("(o r) c -> r o c", o=2))
                        dsum = dp.tile([8, 512], F32, tag="dsum",
                                       name=f"dsum{c}", bufs=1)
                        nc.vector.tensor_add(dsum, dpk[:, 0, :],
                                             dpk[:, 1, :])
                        rpack = dp.tile([8, 512], F16, tag="rpack",
                                        name=f"rpack{c}", bufs=1)
                        nc.vector.reciprocal(out=rpack, in_=dsum)
                        nc.sync.dma_start(out=rden_d[c], in_=rpack)
                        rstrip = dp.tile([1, 8, 512], F16, tag="rstrip",
                                         name=f"rstrip{c}", bufs=1)
                        nc.sync.dma_start(
                            out=rstrip,
                            in_=rden_d[c].rearrange("(o r) c -> o r c", o=1))
                        for hh in (h - 1, h):
                            ct16 = cp.tile([128, S], F16, tag="ctxh",
                                           name=f"ctxh{hh}")
                            for qb in range(NB):
                                row = (hh % 2) * 4 + qb
                                dbc_ps = pp.tile([128, 512], F32, tag="ps1",
                                                 name=f"dbc{hh}_{qb}")
                                nc.tensor.matmul(
                                    dbc_ps, ones_row, rstrip[:, row, :],
                                    start=True, stop=True)
                                nc.vector.tensor_mul(
                                    ct16[:, qb * 512:(qb + 1) * 512],
                                    ctx_u[(hh, qb)], dbc_ps)
                            nc.sync.dma_start(
                                out=ctx_send[c][(hh % 2) * 128:
                                                (hh % 2) * 128 + 128, :],
                                in_=ct16)
                        nc.gpsimd.collective_compute(
                            "AllGather",
                            mybir.AluOpType.bypass,
                            replica_groups=[[0, 1], [2, 3], [4, 5], [6, 7]],
                            ins=[ctx_send[c].opt()],
                            outs=[ctx_recv[c].opt()],
                        )

                # interleaved emission: projections feed attention per head
                qk_proj(0)
                qk_proj(1)
                v_proj(0)
                load_v4(0)
                qk_proj(2)
                attention(0)
                qk_proj(3)
                attention(1)
                v_proj(1)
                qk_proj(4)
                attention(2)
                qk_proj(5)
                attention(3)
                load_v4(1)
                qk_proj(6)
                attention(4)
                qk_proj(7)
                attention(5)
                attention(6)
                attention(7)

            # ---------------- phase 4: output projection ----------------
            korder = []
            for c in range(NCHUNK):
                korder += [(c, 0, 2 * c), (c, 128, 2 * c + 1),
                           (c, 256, 8 + 2 * c), (c, 384, 8 + 2 * c + 1)]
            NA = 12    # strips from AllGather chunks 0-2 (pass A)
            with tc.tile_pool(name="p4wo", bufs=32) as wop, \
                 tc.tile_pool(name="p4ct", bufs=16) as ctp, \
                 tc.tile_pool(name="p4acc", bufs=16) as accp, \
                 tc.tile_pool(name="p4o", bufs=3) as op_, \
                 tc.tile_pool(name="p4b", bufs=1) as bp4, \
                 tc.tile_pool(name="ps4", bufs=4, space="PSUM") as pp4:
                # broadcast bo across partitions via ones outer product
                bo_bc = bp4.tile([128, HHID], F32, tag="bo_bc")
                for n in range(HHID // 512):
                    bps = pp4.tile([128, 512], F32, tag="ps4", name=f"bps{n}")
                    nc.tensor.matmul(bps, ones_row,
                                     bo_sb[:, n * 512:(n + 1) * 512],
                                     start=True, stop=True)
                    nc.vector.tensor_copy(out=bo_bc[:, n * 512:(n + 1) * 512],
                                          in_=bps)
                wo = {}
                ct = []
                for ki, (c, off, gk) in enumerate(korder):
                    t = ctp.tile([128, S], F16, tag="ct", name=f"ct{gk}")
                    nc.sync.dma_start(out=t, in_=ctx_recv[c][off:off + 128, :])
                    ct.append(t)
                    for n in range(HHID // 512):
                        w = wop.tile([128, 512], F16, tag="wo",
                                     name=f"wo{gk}_{n}")
                        nc.sync.dma_start(
                            out=w,
                            in_=woT[gk * 128:(gk + 1) * 128,
                                    n * 512:(n + 1) * 512])
                        wo[(ki, n)] = w
                # pass A: strips from chunks 0-2 (+bias) into SBUF while the
                # last AllGather is still in flight
                accs = []
                for m in range(S // 128):
                    acc = accp.tile([128, HHID], F32, tag="acc",
                                    name=f"acc{m}")
                    for n in range(HHID // 512):
                        ps = pp4.tile([128, 512], F32, tag="ps4",
                                      name=f"psA_{m}_{n}")
                        for ki in range(NA):
                            nc.tensor.matmul(
                                ps, ct[ki][:, m * 128:(m + 1) * 128],
                                wo[(ki, n)],
                                start=(ki == 0), stop=(ki == NA - 1))
                        nc.vector.tensor_add(
                            acc[:, n * 512:(n + 1) * 512], ps,
                            bo_bc[:, n * 512:(n + 1) * 512])
                    accs.append(acc)
                # pass B: last chunk's 4 strips + accumulated partials
                for m in range(S // 128):
                    osb = op_.tile([128, HHID], F32, tag="osb", name=f"osb{m}")
                    for n in range(HHID // 512):
                        ps = pp4.tile([128, 512], F32, tag="ps4",
                                      name=f"psB_{m}_{n}")
                        for ki in range(NA, KT):
                            nc.tensor.matmul(
                                ps, ct[ki][:, m * 128:(m + 1) * 128],
                                wo[(ki, n)],
                                start=(ki == NA), stop=(ki == KT - 1))
                        nc.vector.tensor_add(
                            osb[:, n * 512:(n + 1) * 512], ps,
                            accs[m][:, n * 512:(n + 1) * 512])
                    nc.sync.dma_start(out=out[m * 128:(m + 1) * 128, :],
                                      in_=osb)

    nc.compile()
    return nc


def _get_nc():
    if "nc" not in _CACHED:
        _CACHED["nc"] = _build_program()
    return _CACHED["nc"]


def _make_masks():
    i = np.arange(128)[:, None]
    j = np.arange(512)[None, :]
    return np.stack(
        [((j - i) >= 128 * r).astype(np.float16) for r in range(4)], axis=0)


def _make_in_maps(inputs):
    x = np.ascontiguousarray(np.asarray(inputs["x"], dtype=np.float32))
    Wq = np.asarray(inputs["Wq"], dtype=np.float32)
    Wk = np.asarray(inputs["Wk"], dtype=np.float32)
    Wv = np.asarray(inputs["Wv"], dtype=np.float32)
    Wo = np.asarray(inputs["Wo"], dtype=np.float32)
    bq = np.asarray(inputs["bq"], dtype=np.float32)
    bk = np.asarray(inputs["bk"], dtype=np.float32)
    bv = np.asarray(inputs["bv"], dtype=np.float32)
    bo = np.asarray(inputs["bo"], dtype=np.float32)

    bo_eff = bo + Wo @ bv
    masks = _make_masks()
    WqT = np.ascontiguousarray(Wq.T)
    WkT = np.ascontiguousarray(Wk.T)
    WvT = np.ascontiguousarray(Wv.T)
    WoT = np.ascontiguousarray(Wo.T)

    in_maps = []
    for c in range(N_CORES):
        b, hf = c // 2, c % 2
        sl = slice(hf * HHID, (hf + 1) * HHID)
        in_maps.append({
            "xT": np.ascontiguousarray(x[b].T).astype(np.float16),
            "wqT": np.ascontiguousarray(WqT[:, sl]).astype(np.float16),
            "wkT": np.ascontiguousarray(WkT[:, sl]).astype(np.float16),
            "wvT": np.ascontiguousarray(WvT[:, sl]).astype(np.float16),
            "woT": np.ascontiguousarray(WoT[:, sl]).astype(np.float16),
            "bq": np.ascontiguousarray(bq[sl].reshape(HH, 128).T),
            "bk": np.ascontiguousarray(bk[sl].reshape(HH, 128).T),
            "bo": bo_eff[sl].reshape(1, HHID).astype(np.float16),
            "masks": masks,
        })
    return in_maps


def kernel(**inputs):
    from concourse.bass_utils import run_bass_kernel_spmd

    in_maps = _make_in_maps(inputs)
    nc = _get_nc()
    res = run_bass_kernel_spmd(nc, in_maps, list(range(N_CORES)))

    out = np.empty((B, S, HID), dtype=np.float32)
    for c in range(N_CORES):
        b, hf = c // 2, c % 2
        out[b, :, hf * HHID:(hf + 1) * HHID] = res.results[c]["out"]
    return out


# revision 9
# speedup vs baseline: 1.1859x; 1.0107x over previous
"""Causal self-attention (B=4, S=2048, H=2048, 16 heads) on 8 Trainium2 NeuronCores.

Sharding: DP4 over batch x TP2 over heads. Core c handles batch c//2 and head
half c%2 (8 heads of 128 dims). fp16 matmul operands throughout (PSUM always
accumulates fp32). Per core:
  phase 1: V ([s,d] layout) and Q^T,K^T ([d,s] layout) projections, bounced to
           DRAM scratch (fp16). All weight/x loads are BATCHED multi-dim-AP
           DMAs (one trigger per head-projection / x-chunk) because the Sync
           engine serializes triggers at ~600ns each; head-0 weights are
           issued before the x tiles.
  phase 2: per head, causal flash-style attention in the transposed layout
           (scores^T [k,q]). Per 2-tile pair: scores -> exp in two 512-wide
           ACTIVATEs -> mask -> [next pair's scores] -> ctx, ctx, den-pair
           (ones-matmuls packed into PE col-groups 0/64, concurrent).
           Tiles strictly above the diagonal (r>=1) trim their q-range to
           [128r, 512) in the score/exp/mask/ctx/den ops. Unnormalized ctx^T
           parked in SBUF fp16; per-head denominator chain (DRAM repack of
           even/odd partials, add, batched reciprocal, PE-broadcast,
           normalize) runs right after each head so only head 7's short
           chain is exposed at the end.
  phase 3: four pairwise fp16 AllGathers (one per 2-head group), launched as
           each group finishes so they overlap the remaining attention.
  phase 4: fp16 output projection from 16 ctx row-strips, two passes: pass A
           accumulates the 12 strips from AllGather chunks 0-2 (+bias) into
           SBUF while chunk 3 is still in flight; pass B adds the last 4
           strips. Bias folded on host (bo_eff = bo + Wo @ bv).
Host assembles out[b, :, o_half] = per-core out [s, o_half].
"""

import math
import sys

if "/opt/trn_rl_repo" not in sys.path:
    sys.path.insert(0, "/opt/trn_rl_repo")

import numpy as np

B, S, HID = 4, 2048, 2048
HEADS, D = 16, 128
HH = HEADS // 2          # heads per core
HHID = HH * D            # 1024, per-core head-span of hidden
KT = HID // 128          # 16 contraction tiles of 128
NB = S // 512            # 4 free-dim blocks of 512
N_CORES = 8
NCHUNK = 4               # ctx-exchange chunks (2 heads each)

_CACHED = {}


def _build_program():
    import concourse.tile as tile
    import concourse.mybir as mybir
    from concourse import bacc
    from concourse._compat import get_trn_type

    F32 = mybir.dt.float32
    F16 = mybir.dt.float16
    Exp = mybir.ActivationFunctionType.Exp

    nc = bacc.Bacc(
        get_trn_type() or "TRN2",
        target_bir_lowering=False,
        debug=False,
        enable_asserts=False,
        num_devices=N_CORES,
    )

    def din(name, shape, dt=F16):
        return nc.dram_tensor(name, shape, dt, kind="ExternalInput").ap()

    xT = din("xT", [HID, S])          # x[b].T, fp16
    wqT = din("wqT", [HID, HHID])     # Wq.T columns for this core's heads
    wkT = din("wkT", [HID, HHID])
    wvT = din("wvT", [HID, HHID])
    woT = din("woT", [HID, HHID])     # Wo.T columns for this core's o-half
    bq = din("bq", [128, HH], F32)    # bq[h*128+p] at [p, h]
    bk = din("bk", [128, HH], F32)
    bo = din("bo", [1, HHID], F16)    # bo_eff slice for this core's o-half
    masks = din("masks", [4, 128, 512])
    out = nc.dram_tensor("out", [S, HHID], F32, kind="ExternalOutput").ap()

    inv_sqrt_d = float(1.0 / math.sqrt(D))

    with tile.TileContext(nc) as tc, \
         nc.allow_low_precision(reason="fp16 operand pipeline"):
        with tc.tile_pool(name="const", bufs=1) as constp, \
             tc.tile_pool(name="dram", bufs=1, space="DRAM") as dramp:
            # DRAM scratch (fp16 except denominator strips)
            qTd = dramp.tile([HHID, S], F16, tag="qTd")
            kTd = dramp.tile([HHID, S], F16, tag="kTd")
            vd = dramp.tile([S, HHID], F16, tag="vd")
            # rows 0-7: even-kt partials, rows 8-15: odd-kt partials
            dden_d = [dramp.tile([16, 512], F32, tag=f"dden{c}",
                                 name=f"dden{c}") for c in range(NCHUNK)]
            rden_d = [dramp.tile([8, 512], F16, tag=f"rden{c}",
                                 name=f"rden{c}") for c in range(NCHUNK)]
            ctx_send = [dramp.tile([256, S], F16, tag=f"ctxs{c}",
                                   name=f"ctxs{c}") for c in range(NCHUNK)]
            ctx_recv = [dramp.tile([512, S], F16, tag=f"ctxr{c}",
                                   name=f"ctxr{c}") for c in range(NCHUNK)]

            # constants
            ones_col = constp.tile([128, 1], F16, tag="ones_col")
            nc.vector.memset(ones_col, 1.0)
            ones_row = constp.tile([1, 128], F16, tag="ones_row")
            nc.vector.memset(ones_row, 1.0)
            mask_all = constp.tile([128, 4, 512], F16, tag="masks")
            nc.sync.dma_start(out=mask_all,
                              in_=masks.rearrange("r p c -> p r c"))
            mask_t = [mask_all[:, r, :] for r in range(4)]
            bq_sb = constp.tile([128, HH], F32, tag="bq_sb")
            nc.sync.dma_start(out=bq_sb, in_=bq)
            bk_sb = constp.tile([128, HH], F32, tag="bk_sb")
            nc.sync.dma_start(out=bk_sb, in_=bk)
            bo_sb = constp.tile([1, HHID], F16, tag="bo_sb")
            nc.sync.dma_start(out=bo_sb, in_=bo)

            with tc.tile_pool(name="xk", bufs=2) as xp, \
                 tc.tile_pool(name="p1s", bufs=2) as sp, \
                 tc.tile_pool(name="p1w", bufs=3) as wp, \
                 tc.tile_pool(name="p1v", bufs=3) as vp, \
                 tc.tile_pool(name="p1wv", bufs=1) as wvp, \
                 tc.tile_pool(name="p2qk", bufs=3) as qkp, \
                 tc.tile_pool(name="p2v", bufs=2) as v4p, \
                 tc.tile_pool(name="p2et", bufs=2) as etp, \
                 tc.tile_pool(name="p2cu", bufs=9) as cup, \
                 tc.tile_pool(name="p2c", bufs=2) as cp, \
                 tc.tile_pool(name="p2d", bufs=2) as dp, \
                 tc.tile_pool(name="ps1", bufs=2, space="PSUM") as pp, \
                 tc.tile_pool(name="ps2s", bufs=2, space="PSUM") as pps, \
                 tc.tile_pool(name="ps2c", bufs=1, space="PSUM") as ppc, \
                 tc.tile_pool(name="ps2d", bufs=1, space="PSUM") as ppd:
                # batched weight load: all 16 k-tiles of (proj, head) in one
                # DMA trigger. head-0 q,k first: they gate the first matmuls.
                wqk = {}

                def load_w(pname, h):
                    wT = wqT if pname == "q" else wkT
                    t = wp.tile([128, KT, 128], F16, tag="w",
                                name=f"w{pname}{h}")
                    nc.sync.dma_start(
                        out=t,
                        in_=wT[:, h * 128:(h + 1) * 128].rearrange(
                            "(k p) c -> p k c", p=128))
                    wqk[(pname, h)] = t

                load_w("q", 0)
                load_w("k", 0)

                # x^T halves, chunked 4 k-tiles per DMA trigger
                xall = []
                for half in range(2):
                    t = xp.tile([128, KT, 1024], F16, tag="xk",
                                name=f"x{half}")
                    src = xT[:, half * 1024:(half + 1) * 1024].rearrange(
                        "(k p) c -> p k c", p=128)
                    for j in range(4):
                        nc.sync.dma_start(out=t[:, 4 * j:4 * j + 4, :],
                                          in_=src[:, 4 * j:4 * j + 4, :])
                    xall.append(t)

                def xslice(k, lo, size):
                    half, off = lo // 1024, lo % 1024
                    return xall[half][:, k, off:off + size]

                def v_proj(g):
                    wv = wvp.tile([128, KT, 512], F16, tag="wv",
                                  name=f"wv{g}")
                    nc.sync.dma_start(
                        out=wv,
                        in_=wvT[:, g * 512:(g + 1) * 512].rearrange(
                            "(k p) c -> p k c", p=128))
                    for m in range(KT):
                        ps = pp.tile([128, 512], F32, tag="ps1",
                                     name=f"psv{g}_{m}")
                        for k in range(KT):
                            nc.tensor.matmul(
                                ps, xslice(k, m * 128, 128), wv[:, k, :],
                                start=(k == 0), stop=(k == KT - 1))
                        vsb = vp.tile([128, 512], F16, tag="vout",
                                      name=f"v{g}_{m}")
                        nc.vector.tensor_copy(out=vsb, in_=ps)
                        nc.sync.dma_start(
                            out=vd[m * 128:(m + 1) * 128,
                                   g * 512:(g + 1) * 512], in_=vsb)

                def qk_proj(h):
                    for dst, bias_sb, pname in (
                        (qTd, bq_sb, "q"),
                        (kTd, bk_sb, "k"),
                    ):
                        if (pname, h) not in wqk:
                            load_w(pname, h)
                        wt = wqk.pop((pname, h))
                        osb = sp.tile([128, S], F16, tag="projout",
                                      name=f"o{pname}{h}")
                        for n in range(NB):
                            ps = pp.tile([128, 512], F32, tag="ps1",
                                         name=f"ps{pname}{h}_{n}")
                            for k in range(KT):
                                nc.tensor.matmul(
                                    ps, wt[:, k, :], xslice(k, n * 512, 512),
                                    start=(k == 0), stop=(k == KT - 1))
                            nc.vector.tensor_scalar_add(
                                osb[:, n * 512:(n + 1) * 512], ps,
                                bias_sb[:, h:h + 1])
                        nc.sync.dma_start(
                            out=dst[h * 128:(h + 1) * 128, :], in_=osb)

                v4 = [None, None]
                ctx_u = {}

                def load_v4(g):
                    t = v4p.tile([128, KT, 512], F16, tag="v4",
                                 name=f"v4_{g}")
                    src = vd[:, g * 512:(g + 1) * 512].rearrange(
                        "(k p) c -> p k c", p=128)
                    for j in range(2):
                        nc.sync.dma_start(out=t[:, 8 * j:8 * j + 8, :],
                                          in_=src[:, 8 * j:8 * j + 8, :])
                    v4[g] = t

                def attention(h):
                    g, sub = h // 4, h % 4
                    c = h // 2
                    qh = qkp.tile([128, S], F16, tag="qh", name=f"qh{h}")
                    nc.sync.dma_start(out=qh,
                                      in_=qTd[h * 128:(h + 1) * 128, :])
                    kh = qkp.tile([128, S], F16, tag="kh", name=f"kh{h}")
                    nc.sync.dma_start(out=kh,
                                      in_=kTd[h * 128:(h + 1) * 128, :])

                    state = {}   # per-qb psum tiles

                    def tile_lo(qb, kt):
                        # causal trim: tiles strictly above the diagonal only
                        # need q >= 128*r. kt==1 is the odd den-group's first
                        # matmul (must cover the full row for has_written).
                        r = kt - 4 * qb
                        return 128 * r if (r >= 1 and kt != 1) else 0

                    def emit_scores(qb, kt0):
                        sps = pps.tile([128, 1024], F32, tag="sps",
                                       name=f"sps{h}_{qb}_{kt0}")
                        et = etp.tile([128, 1024], F16, tag="et",
                                      name=f"et{h}_{qb}_{kt0}")
                        for i in range(2):
                            kt = kt0 + i
                            lo = tile_lo(qb, kt)
                            sl = slice(i * 512 + lo, (i + 1) * 512)
                            nc.tensor.matmul(
                                sps[:, sl],
                                kh[:, kt * 128:(kt + 1) * 128],
                                qh[:, qb * 512 + lo:(qb + 1) * 512],
                                start=True, stop=True)
                            nc.scalar.activation(
                                out=et[:, sl], in_=sps[:, sl],
                                func=Exp, scale=inv_sqrt_d)
                            r = kt - 4 * qb
                            if r >= 0:
                                nc.vector.tensor_mul(
                                    et[:, sl], et[:, sl],
                                    mask_t[r][:, lo:])
                        return et

                    def emit_dc(qb, kt0, et, kept):
                        if kt0 == 0:
                            state[qb] = (
                                ppc.tile([128, 512], F32, tag="ctxps",
                                         name=f"cps{h}_{qb}"),
                                ppd.tile([128, 512], F32, tag="denps",
                                         name=f"dps{h}_{qb}"),
                            )
                        ctx_ps, den_ps = state[qb]
                        for i in range(2):
                            kt = kt0 + i
                            lo = tile_lo(qb, kt)
                            nc.tensor.matmul(
                                ctx_ps[:, lo:],
                                v4[g][:, kt, sub * 128:(sub + 1) * 128],
                                et[:, i * 512 + lo:(i + 1) * 512],
                                start=(kt == 0), stop=(kt == kept - 1))
                        for i in range(2):
                            kt = kt0 + i
                            lo = tile_lo(qb, kt)
                            nc.tensor.matmul(
                                den_ps[64 * i:64 * i + 1, lo:],
                                ones_col, et[:, i * 512 + lo:(i + 1) * 512],
                                start=(kt0 == 0), stop=(kt0 == kept - 2),
                                tile_position=(0, 64 * i),
                                skip_group_check=True)
                        if kt0 == kept - 2:
                            # qb epilogue: evacuate den partials + ctx
                            row = (h % 2) * 4 + qb
                            den_sb = dp.tile([65, 512], F32, tag="densb",
                                             name=f"den{h}_{qb}", bufs=2)
                            nc.vector.tensor_copy(out=den_sb,
                                                  in_=den_ps[0:65, :])
                            nc.sync.dma_start(
                                out=dden_d[c][row:row + 1, :],
                                in_=den_sb[0:1, :])
                            nc.sync.dma_start(
                                out=dden_d[c][8 + row:9 + row, :],
                                in_=den_sb[64:65, :])
                            cu = cup.tile([128, 512], F16, tag="cu",
                                          name=f"cu{h}_{qb}")
                            nc.vector.tensor_copy(out=cu, in_=ctx_ps)
                            ctx_u[(h, qb)] = cu

                    units = []
                    for qb in range(NB):
                        kept = min(4 * qb + 4, KT)
                        for kt0 in range(0, kept, 2):
                            units.append((qb, kt0, kept))
                    prev = None
                    for qb, kt0, kept in units:
                        et = emit_scores(qb, kt0)
                        if prev is not None:
                            emit_dc(*prev)
                        prev = (qb, kt0, et, kept)
                    emit_dc(*prev)

                    # per-head denominator chain + normalize, so only head
                    # 7's short chain is exposed at the kernel end
                    hr = (h % 2) * 4
                    dpk = dp.tile([4, 2, 512], F32, tag="dpack",
                                  name=f"dpack{h}", bufs=1)
                    nc.sync.dma_start(
                        out=dpk,
                        in_=dden_d[c].rearrange(
                            "(o r) c -> r o c", o=2)[hr:hr + 4, :, :])
                    dsum = dp.tile([4, 512], F32, tag="dsum",
                                   name=f"dsum{h}", bufs=1)
                    nc.vector.tensor_add(dsum, dpk[:, 0, :], dpk[:, 1, :])
                    rpack = dp.tile([4, 512], F16, tag="rpack",
                                    name=f"rpack{h}", bufs=1)
                    nc.vector.reciprocal(out=rpack, in_=dsum)
                    nc.sync.dma_start(out=rden_d[c][hr:hr + 4, :], in_=rpack)
                    rstrip = dp.tile([1, 4, 512], F16, tag="rstrip",
                                     name=f"rstrip{h}", bufs=1)
                    nc.sync.dma_start(
                        out=rstrip,
                        in_=rden_d[c][hr:hr + 4, :].rearrange(
                            "(o r) c -> o r c", o=1))
                    ct16 = cp.tile([128, S], F16, tag="ctxh",
                                   name=f"ctxh{h}")
                    for qb in range(NB):
                        dbc_ps = pp.tile([128, 512], F32, tag="ps1",
                                         name=f"dbc{h}_{qb}")
                        nc.tensor.matmul(
                            dbc_ps, ones_row, rstrip[:, qb, :],
                            start=True, stop=True)
                        nc.vector.tensor_mul(
                            ct16[:, qb * 512:(qb + 1) * 512],
                            ctx_u[(h, qb)], dbc_ps)
                    nc.sync.dma_start(
                        out=ctx_send[c][(h % 2) * 128:(h % 2) * 128 + 128, :],
                        in_=ct16)
                    if h % 2 == 1:
                        nc.gpsimd.collective_compute(
                            "AllGather",
                            mybir.AluOpType.bypass,
                            replica_groups=[[0, 1], [2, 3], [4, 5], [6, 7]],
                            ins=[ctx_send[c].opt()],
                            outs=[ctx_recv[c].opt()],
                        )

                # interleaved emission: projections feed attention per head
                qk_proj(0)
                qk_proj(1)
                v_proj(0)
                load_v4(0)
                qk_proj(2)
                attention(0)
                qk_proj(3)
                attention(1)
                v_proj(1)
                qk_proj(4)
                attention(2)
                qk_proj(5)
                attention(3)
                load_v4(1)
                qk_proj(6)
                attention(4)
                qk_proj(7)
                attention(5)
                attention(6)
                attention(7)

            # ---------------- phase 4: output projection ----------------
            # strip order: chunk-major so chunk 3's strips come last
            korder = []
            for c in range(NCHUNK):
                korder += [(c, 0, 2 * c), (c, 1, 2 * c + 1),
                           (c, 2, 8 + 2 * c), (c, 3, 8 + 2 * c + 1)]
            NA = 12    # strips from AllGather chunks 0-2 (pass A)
            with tc.tile_pool(name="p4wo", bufs=2) as wop, \
                 tc.tile_pool(name="p4ct", bufs=4) as ctp, \
                 tc.tile_pool(name="p4acc", bufs=16) as accp, \
                 tc.tile_pool(name="p4o", bufs=3) as op_, \
                 tc.tile_pool(name="p4b", bufs=1) as bp4, \
                 tc.tile_pool(name="ps4", bufs=4, space="PSUM") as pp4:
                # broadcast bo across partitions via ones outer product
                bo_bc = bp4.tile([128, HHID], F32, tag="bo_bc")
                for n in range(HHID // 512):
                    bps = pp4.tile([128, 512], F32, tag="ps4", name=f"bps{n}")
                    nc.tensor.matmul(bps, ones_row,
                                     bo_sb[:, n * 512:(n + 1) * 512],
                                     start=True, stop=True)
                    nc.vector.tensor_copy(out=bo_bc[:, n * 512:(n + 1) * 512],
                                          in_=bps)
                # batched loads: one DMA per AllGather chunk (4 strips) and
                # one per wo column-half (16 strips)
                woall = []
                for n in range(HHID // 512):
                    t = wop.tile([128, KT, 512], F16, tag="wo",
                                 name=f"wo{n}")
                    nc.sync.dma_start(
                        out=t,
                        in_=woT[:, n * 512:(n + 1) * 512].rearrange(
                            "(g p) c -> p g c", p=128))
                    woall.append(t)
                ctall = []
                for c in range(NCHUNK):
                    t = ctp.tile([128, 4, S], F16, tag="ct", name=f"ct{c}")
                    nc.sync.dma_start(
                        out=t,
                        in_=ctx_recv[c].rearrange("(s p) c -> p s c", p=128))
                    ctall.append(t)

                def ct_ap(ki, m):
                    c, s, gk = korder[ki]
                    return ctall[c][:, s, m * 128:(m + 1) * 128]

                def wo_ap(ki, n):
                    gk = korder[ki][2]
                    return woall[n][:, gk, :]

                # pass A: strips from chunks 0-2 (+bias) into SBUF while the
                # last AllGather is still in flight
                accs = []
                for m in range(S // 128):
                    acc = accp.tile([128, HHID], F32, tag="acc",
                                    name=f"acc{m}")
                    for n in range(HHID // 512):
                        ps = pp4.tile([128, 512], F32, tag="ps4",
                                      name=f"psA_{m}_{n}")
                        for ki in range(NA):
                            nc.tensor.matmul(
                                ps, ct_ap(ki, m), wo_ap(ki, n),
                                start=(ki == 0), stop=(ki == NA - 1))
                        nc.vector.tensor_add(
                            acc[:, n * 512:(n + 1) * 512], ps,
                            bo_bc[:, n * 512:(n + 1) * 512])
                    accs.append(acc)
                # pass B: last chunk's 4 strips + accumulated partials
                for m in range(S // 128):
                    osb = op_.tile([128, HHID], F32, tag="osb", name=f"osb{m}")
                    for n in range(HHID // 512):
                        ps = pp4.tile([128, 512], F32, tag="ps4",
                                      name=f"psB_{m}_{n}")
                        for ki in range(NA, KT):
                            nc.tensor.matmul(
                                ps, ct_ap(ki, m), wo_ap(ki, n),
                                start=(ki == NA), stop=(ki == KT - 1))
                        nc.vector.tensor_add(
                            osb[:, n * 512:(n + 1) * 512], ps,
                            accs[m][:, n * 512:(n + 1) * 512])
                    nc.sync.dma_start(out=out[m * 128:(m + 1) * 128, :],
                                      in_=osb)

    nc.compile()
    return nc


def _get_nc():
    if "nc" not in _CACHED:
        _CACHED["nc"] = _build_program()
    return _CACHED["nc"]


def _make_masks():
    i = np.arange(128)[:, None]
    j = np.arange(512)[None, :]
    return np.stack(
        [((j - i) >= 128 * r).astype(np.float16) for r in range(4)], axis=0)


def _make_in_maps(inputs):
    x = np.ascontiguousarray(np.asarray(inputs["x"], dtype=np.float32))
    Wq = np.asarray(inputs["Wq"], dtype=np.float32)
    Wk = np.asarray(inputs["Wk"], dtype=np.float32)
    Wv = np.asarray(inputs["Wv"], dtype=np.float32)
    Wo = np.asarray(inputs["Wo"], dtype=np.float32)
    bq = np.asarray(inputs["bq"], dtype=np.float32)
    bk = np.asarray(inputs["bk"], dtype=np.float32)
    bv = np.asarray(inputs["bv"], dtype=np.float32)
    bo = np.asarray(inputs["bo"], dtype=np.float32)

    bo_eff = bo + Wo @ bv
    masks = _make_masks()
    WqT = np.ascontiguousarray(Wq.T)
    WkT = np.ascontiguousarray(Wk.T)
    WvT = np.ascontiguousarray(Wv.T)
    WoT = np.ascontiguousarray(Wo.T)

    in_maps = []
    for c in range(N_CORES):
        b, hf = c // 2, c % 2
        sl = slice(hf * HHID, (hf + 1) * HHID)
        in_maps.append({
            "xT": np.ascontiguousarray(x[b].T).astype(np.float16),
            "wqT": np.ascontiguousarray(WqT[:, sl]).astype(np.float16),
            "wkT": np.ascontiguousarray(WkT[:, sl]).astype(np.float16),
            "wvT": np.ascontiguousarray(WvT[:, sl]).astype(np.float16),
            "woT": np.ascontiguousarray(WoT[:, sl]).astype(np.float16),
            "bq": np.ascontiguousarray(bq[sl].reshape(HH, 128).T),
            "bk": np.ascontiguousarray(bk[sl].reshape(HH, 128).T),
            "bo": bo_eff[sl].reshape(1, HHID).astype(np.float16),
            "masks": masks,
        })
    return in_maps


def kernel(**inputs):
    from concourse.bass_utils import run_bass_kernel_spmd

    in_maps = _make_in_maps(inputs)
    nc = _get_nc()
    res = run_bass_kernel_spmd(nc, in_maps, list(range(N_CORES)))

    out = np.empty((B, S, HID), dtype=np.float32)
    for c in range(N_CORES):
        b, hf = c // 2, c % 2
        out[b, :, hf * HHID:(hf + 1) * HHID] = res.results[c]["out"]
    return out
